# revision 1
# baseline (speedup 1.0000x reference)
"""BitNet 4-layer MLP (8192x4096, ternary weights, int8-style activations)
on 8 Trainium2 NeuronCores.

Strategy: pure data-parallel over the 8192-token dim (1024 tokens/core, no
collectives). Activations live TRANSPOSED on chip ([feature, token]) so the
output of each layer's matmul (PSUM [out_feat, tok]) is directly the next
layer's moving operand — zero transposes on device. Weights are quantized
to ternary bf16 on the host (matmul over {-1,0,1} x integers <= 128 is
exact in bf16 with fp32 PSUM accumulation) and streamed per layer.
LayerNorm stats are partition-dim sums done as ones-vector matmuls in
float32r (full PE rate at N=512). Quantization uses the +/-1.5*2^23
magic-number trick, which matches XLA's round-nearest-even bitwise.

Each core processes its 1024 tokens as two 512-token halves pipelined
against each other: while half B's matmuls run on PE, half A's layernorm/
quantize chain runs on DVE/ACT/GPSIMD, so PE never waits on phase C.
"""

import numpy as np

NUM_CORES = 8
N_TOK, D = 8192, 4096
NUM_LAYERS = 4
P = 128                      # SBUF partitions
KT = D // P                  # 32 k-tiles per contraction
NLOC = N_TOK // NUM_CORES    # 1024 tokens per core
HALF = 512                   # token half-chunk (one PSUM bank @ fp32)
MAGIC = 12582912.0           # 1.5 * 2**23: fp32 add/sub does RNE-to-integer

_prog_cache = {}


def _install_drain_patch():
    """walrus CoreV3 rejects instructions carrying >~2 embedded sem waits
    ("Too many sync wait commands"). Tile's exit drain waits on the whole
    vector clock; spread its waits across trailing sync-engine nops."""
    import concourse.tile as tile
    import concourse.mybir as mybir
    from concourse.tile import ScopedClock

    if getattr(tile.TileContext, "_drain_patch_installed", False):
        return

    def _patched(self, tick_clock, wait_clock):
        nc = self.nc
        drain_inst = nc.sync.drain()
        wait_clock.add_sem_waits(
            drain_inst.ins, ScopedClock({None: tick_clock.global_clock})
        )
        si = drain_inst.ins.sync_info
        waits = list(si.on_wait or []) if si is not None else []
        if len(waits) > 1:
            si.on_wait = waits[:1]
            for w in waits[1:]:
                nop = nc.sync.nop(nofuse=True)
                nsi = nop.ins.sync_info
                if nsi is None:
                    nop.ins.sync_info = mybir.SyncInfo(on_wait=[w], on_update=[])
                else:
                    nsi.on_wait = [w]
        nc.all_engine_barrier()
        assert self.sems is not None
        popped = nc._tile_sem_poison_stack.pop()
        assert popped is self._sem_poison
        nc.clear_and_free_semaphores(list(self.sems.allocated().values()))
        nc.all_engine_barrier()

    tile.TileContext._drain_and_barrier = _patched
    tile.TileContext._drain_patch_installed = True


def _split_excess_waits(nc, maxw=1):
    """walrus's per-instruction sync-wait encodings hold few waits; hoist
    excess waits onto same-engine nops spliced immediately before the
    overloaded instruction (adjacent on the same queue, so ordering
    semantics are unchanged)."""
    import copy
    import concourse.mybir as mybir

    ctr = [0]
    # a genuine InstNoOp prototype (left at stream end, harmless)
    proto = nc.sync.nop(nofuse=True)
    _NOP_PROTO = copy.deepcopy(proto.ins)
    _NOP_PROTO.sync_info = None

    def make_nop(proto_engine, waits):
        ctr[0] += 1
        nop = copy.deepcopy(_NOP_PROTO)
        nop.name = f"I-waitsplit-{ctr[0]}"
        nop.engine = proto_engine
        nop.sync_info = mybir.SyncInfo(on_wait=list(waits), on_update=[])
        return nop

    for bb in nc.m.functions[0].blocks:
        changed = False
        out = []
        for inst in bb.instructions:
            si = inst.sync_info
            waits = list(si.on_wait) if (si is not None and si.on_wait) else []
            if len(waits) > maxw and type(inst).__name__ != "InstISA":
                for i in range(0, len(waits) - maxw, maxw):
                    out.append(make_nop(inst.engine, waits[i:i + maxw]))
                si.on_wait = waits[len(waits) - maxw:]
                changed = True
            out.append(inst)
        if changed:
            bb.instructions = out


def _build_program(s_deq, inv_in):
    """Build the per-core Bass program (identical across cores; data-parallel).

    s_deq[l]  = in_scale[l]*w_scale[l] as python floats (fp32-exact values)
    inv_in[l] = 1/in_scale[l] likewise
    """
    import concourse.bass as bass
    import concourse.mybir as mybir
    import concourse.tile as tile

    _install_drain_patch()
    dt = mybir.dt
    Alu = mybir.AluOpType
    Act = mybir.ActivationFunctionType

    nc = bass.Bass()
    W_d = nc.dram_tensor("wt", [NUM_LAYERS, KT, P, KT, P], dt.bfloat16,
                         kind="ExternalInput")
    X_d = nc.dram_tensor("xq0", [KT, P, NLOC], dt.bfloat16, kind="ExternalInput")
    G_d = nc.dram_tensor("gam", [NUM_LAYERS - 1, KT, P, 1], dt.float32,
                         kind="ExternalInput")
    B_d = nc.dram_tensor("bet", [NUM_LAYERS - 1, KT, P, 1], dt.float32,
                         kind="ExternalInput")
    O_d = nc.dram_tensor("out", [D, NLOC], dt.float32, kind="ExternalOutput")

    f32, f32r, bf16 = dt.float32, dt.float32r, dt.bfloat16

    with tile.TileContext(nc) as tc:
        with (
            tc.tile_pool(name="xq", bufs=64) as xq_pool,
            tc.tile_pool(name="h", bufs=33) as h_pool,
            tc.tile_pool(name="w", bufs=2) as w_pool,
            tc.tile_pool(name="sq", bufs=2) as sq_pool,
            tc.tile_pool(name="hr", bufs=2) as hr_pool,
            tc.tile_pool(name="bc", bufs=4) as bc_pool,
            tc.tile_pool(name="st", bufs=4) as st_pool,
            tc.tile_pool(name="gb", bufs=128) as gb_pool,
            tc.tile_pool(name="const", bufs=1) as const_pool,
            tc.tile_pool(name="mmps", bufs=2, space="PSUM") as mm_ps,
            tc.tile_pool(name="stps", bufs=4, space="PSUM") as st_ps,
            tc.tile_pool(name="bcps", bufs=2, space="PSUM") as bc_ps,
        ):
            ones_f = const_pool.tile([P, 1], f32)
            nc.vector.memset(ones_f[:], 1.0)
            ones = const_pool.tile([P, 1], f32r)
            nc.vector.tensor_copy(ones[:], ones_f[:])
            eps = const_pool.tile([1, 1], f32)
            nc.vector.memset(eps[:], 1e-5)
            ones_row = const_pool.tile([1, P], f32)
            nc.vector.memset(ones_row[:], 1.0)

            xq_tiles = {}
            for half in range(2):
                for kt in range(KT):
                    t = xq_pool.tile([P, HALF], bf16, tag="xq")
                    nc.sync.dma_start(
                        t[:], X_d[kt, :, half * HALF:(half + 1) * HALF])
                    xq_tiles[(0, half, kt)] = t

            h_tiles = {}

            def emit_mm(l, half):
                if l < NUM_LAYERS - 1:
                    S_ps = st_ps.tile([1, HALF], f32, tag="stps")
                    Q_ps = st_ps.tile([1, HALF], f32, tag="stps")
                else:
                    S_ps = Q_ps = None
                for ot in range(KT):
                    w = w_pool.tile([P, KT, P], bf16, tag="w")
                    nc.sync.dma_start(w[:], W_d[l, ot])
                    ps = mm_ps.tile([P, HALF], f32, tag="mmps")
                    for kt in range(KT):
                        nc.tensor.matmul(
                            ps[:], w[:, kt, :], xq_tiles[(l, half, kt)][:],
                            start=(kt == 0), stop=(kt == KT - 1),
                            skip_group_check=True)
                    h_t = h_pool.tile([P, HALF], f32, tag="h")
                    if l < NUM_LAYERS - 1:
                        nc.scalar.activation(h_t[:], ps[:], Act.Relu,
                                             scale=float(s_deq[l]))
                        # f32r hi/lo compensation: sum(h) and sum(h^2) at
                        # full fp32 accuracy via two f32r matmuls each
                        h_r = hr_pool.tile([P, HALF], f32r, tag="hr")
                        nc.vector.tensor_copy(h_r[:], h_t[:])
                        h_lo = hr_pool.tile([P, HALF], f32r, tag="hlo")
                        nc.vector.tensor_tensor(h_lo[:], h_t[:],
                                                h_r[:].bitcast(f32),
                                                op=Alu.subtract)
                        nc.tensor.matmul(
                            S_ps[:], ones[:], h_r[:],
                            start=(ot == 0), stop=False,
                            skip_group_check=True)
                        nc.tensor.matmul(
                            S_ps[:], ones[:], h_lo[:],
                            start=False, stop=(ot == KT - 1),
                            skip_group_check=True)
                        sqf = sq_pool.tile([P, HALF], f32, tag="sqf")
                        nc.vector.tensor_tensor(sqf[:], h_t[:], h_t[:],
                                                op=Alu.mult)
                        sqt = sq_pool.tile([P, HALF], f32r, tag="sq")
                        nc.vector.tensor_copy(sqt[:], sqf[:])
                        sq_lo = sq_pool.tile([P, HALF], f32r, tag="sqlo")
                        nc.vector.tensor_tensor(sq_lo[:], sqf[:],
                                                sqt[:].bitcast(f32),
                                                op=Alu.subtract)
                        nc.tensor.matmul(
                            Q_ps[:], ones[:], sqt[:],
                            start=(ot == 0), stop=False,
                            skip_group_check=True)
                        nc.tensor.matmul(
                            Q_ps[:], ones[:], sq_lo[:],
                            start=False, stop=(ot == KT - 1),
                            skip_group_check=True)
                        h_tiles[(half, ot)] = h_t
                    else:
                        nc.scalar.activation(h_t[:], ps[:], Act.Copy,
                                             scale=float(s_deq[l]))
                        nc.sync.dma_start(
                            O_d[ot * P:(ot + 1) * P,
                                half * HALF:(half + 1) * HALF], h_t[:])
                return S_ps, Q_ps

            def emit_post(l, half, S_ps, Q_ps, gams, bets):
                # stats rows [1, HALF] on partition 0
                mu = st_pool.tile([1, HALF], f32, tag="st")
                nc.vector.tensor_scalar_mul(mu[:], S_ps[:], 1.0 / D)
                q = st_pool.tile([1, HALF], f32, tag="st")
                nc.vector.tensor_scalar_mul(q[:], Q_ps[:], 1.0 / D)
                var = st_pool.tile([1, HALF], f32, tag="st")
                nc.vector.tensor_tensor(var[:], mu[:], mu[:], op=Alu.mult)
                nc.vector.tensor_tensor(var[:], q[:], var[:], op=Alu.subtract)
                std = st_pool.tile([1, HALF], f32, tag="st")
                nc.scalar.activation(std[:], var[:], Act.Sqrt, bias=eps[:])
                rstd = st_pool.tile([1, HALF], f32, tag="st")
                nc.vector.reciprocal(rstd[:], std[:])
                mu_ps = bc_ps.tile([P, HALF], f32, tag="bcps")
                nc.tensor.matmul(mu_ps[:], ones_row[:], mu[:],
                                 start=True, stop=True, skip_group_check=True)
                muB = bc_pool.tile([P, HALF], f32, tag="bc")
                nc.scalar.activation(muB[:], mu_ps[:], Act.Copy)
                rstd_ps = bc_ps.tile([P, HALF], f32, tag="bcps")
                nc.tensor.matmul(rstd_ps[:], ones_row[:], rstd[:],
                                 start=True, stop=True, skip_group_check=True)
                rstdB = bc_pool.tile([P, HALF], f32, tag="bc")
                nc.scalar.activation(rstdB[:], rstd_ps[:], Act.Copy)
                inv = float(inv_in[l + 1])
                for ft in range(KT):
                    h_t = h_tiles.pop((half, ft))
                    nc.vector.tensor_tensor(h_t[:], h_t[:], muB[:],
                                            op=Alu.subtract)
                    nc.vector.tensor_tensor(h_t[:], h_t[:], rstdB[:],
                                            op=Alu.mult)
                    nc.vector.tensor_scalar(h_t[:], h_t[:], gams[ft][:],
                                            bets[ft][:], op0=Alu.mult,
                                            op1=Alu.add)
                    nc.vector.tensor_scalar(h_t[:], h_t[:], inv, MAGIC,
                                            op0=Alu.mult, op1=Alu.add)
                    nc.vector.tensor_scalar(h_t[:], h_t[:], MAGIC + 127.0,
                                            MAGIC - 128.0, op0=Alu.min,
                                            op1=Alu.max)
                    xq_t = xq_pool.tile([P, HALF], bf16, tag="xq")
                    nc.vector.tensor_scalar_add(xq_t[:], h_t[:], -MAGIC)
                    xq_tiles[(l + 1, half, ft)] = xq_t

            for l in range(NUM_LAYERS):
                if l < NUM_LAYERS - 1:
                    gams, bets = [], []
                    for ft in range(KT):
                        g = gb_pool.tile([P, 1], f32, tag="gb")
                        nc.sync.dma_start(g[:], G_d[l, ft])
                        gams.append(g)
                        b = gb_pool.tile([P, 1], f32, tag="gb")
                        nc.sync.dma_start(b[:], B_d[l, ft])
                        bets.append(b)
                for half in range(2):
                    S_ps, Q_ps = emit_mm(l, half)
                    if l < NUM_LAYERS - 1:
                        emit_post(l, half, S_ps, Q_ps, gams, bets)

    _split_excess_waits(nc)
    return nc


def kernel(x, Ws, w_scales, in_scales, gammas, betas, _trace=False):
    import ml_dtypes
    from concourse.bass_utils import run_bass_kernel_spmd

    f32 = np.float32
    C = f32(MAGIC)
    x = np.asarray(x, f32)
    Ws = np.asarray(Ws, f32)
    w_scales = np.asarray(w_scales, f32)
    in_scales = np.asarray(in_scales, f32)
    gammas = np.asarray(gammas, f32)
    betas = np.asarray(betas, f32)

    # ---- host prep (offline-weight-style preprocessing) ----
    # ternary quantize weights; XLA divides by reciprocal-multiply and
    # rounds nearest-even, both reproduced here bitwise.
    WT = np.empty((NUM_LAYERS, KT, P, KT, P), ml_dtypes.bfloat16)
    for l in range(NUM_LAYERS):
        wq = ((Ws[l] * (f32(1.0) / w_scales[l])) + C) - C
        wq = np.clip(wq, -1.0, 1.0).astype(f32)
        # WT[l, ot, kp, kt, o] = wq[ot*128+o, kt*128+kp]
        t = wq.reshape(KT, P, KT, P)          # [ot, o, kt, kp]
        WT[l] = t.transpose(0, 3, 2, 1).astype(ml_dtypes.bfloat16)

    xq0 = ((x * (f32(1.0) / in_scales[0])) + C) - C
    xq0 = np.clip(xq0, -128.0, 127.0).astype(f32)
    xT = np.ascontiguousarray(xq0.T)           # [k, n]

    G = np.ascontiguousarray(gammas.reshape(NUM_LAYERS - 1, KT, P, 1))
    B = np.ascontiguousarray(betas.reshape(NUM_LAYERS - 1, KT, P, 1))

    s_deq = [float(in_scales[l] * w_scales[l]) for l in range(NUM_LAYERS)]
    inv_in = [float(f32(1.0) / in_scales[l]) for l in range(NUM_LAYERS)]

    key = (tuple(s_deq), tuple(inv_in))
    if key not in _prog_cache:
        _prog_cache[key] = _build_program(s_deq, inv_in)
    nc = _prog_cache[key]

    in_maps = []
    for c in range(NUM_CORES):
        xs = xT[:, c * NLOC:(c + 1) * NLOC].reshape(KT, P, NLOC)
        in_maps.append({
            "wt": WT,
            "xq0": np.ascontiguousarray(xs).astype(ml_dtypes.bfloat16),
            "gam": G,
            "bet": B,
        })

    res = run_bass_kernel_spmd(nc, in_maps, list(range(NUM_CORES)),
                               trace=_trace)
    if _trace:
        kernel.last_exec_ns = res.exec_time_ns

    outT = np.concatenate(
        [res.results[c]["out"] for c in range(NUM_CORES)], axis=1)
    return np.ascontiguousarray(outT.T).astype(np.float32)


kernel.last_exec_ns = None



# revision 5
# speedup vs baseline: 1.1419x; 1.1419x over previous
"""BitNet 4-layer MLP (8192x4096, ternary weights, int8-style activations)
on 8 Trainium2 NeuronCores.

Strategy: pure data-parallel over the 8192-token dim (1024 tokens/core, no
collectives). Activations live TRANSPOSED on chip ([feature, token]) so the
output of each layer's matmul (PSUM [out_feat, tok]) is directly the next
layer's moving operand — zero transposes on device. Weights are quantized
to ternary bf16 on the host (matmul over {-1,0,1} x integers <= 128 is
exact in bf16 with fp32 PSUM accumulation) and streamed per layer.

v2 vs baseline: LayerNorm stats no longer ride the PE per output tile.
The 32 h tiles of a half are accumulated on DVE (sum and sum-of-squares),
and a single ones-vector matmul per stat does the final 128-partition
reduction — 12 stats matmuls total instead of 768. Stats/broadcast
matmuls for a finished half are emitted a few tiles into the NEXT half's
main matmuls so the in-order PE queue never stalls on DVE row math.
Weight DMA is prefetched 3 tiles deep (kills the per-phase 2.4us gap),
gamma/beta load as one strided DMA per layer, and the quantize chain is
fused to 7 DVE ops per tile using the +/-1.5*2^23 magic-number round.

Each core processes its 1024 tokens as two 512-token halves pipelined
against each other: while half B's matmuls run on PE, half A's layernorm/
quantize chain runs on DVE, so PE never waits.
"""

import numpy as np

NUM_CORES = 8
N_TOK, D = 8192, 4096
NUM_LAYERS = 4
P = 128                      # SBUF partitions
KT = D // P                  # 32 k-tiles per contraction
NLOC = N_TOK // NUM_CORES    # 1024 tokens per core
HALF = 512                   # token half-chunk (one PSUM bank @ fp32)
MAGIC = 12582912.0           # 1.5 * 2**23: fp32 add/sub does RNE-to-integer

_prog_cache = {}


def _install_drain_patch():
    """walrus CoreV3 rejects instructions carrying >~2 embedded sem waits
    ("Too many sync wait commands"). Tile's exit drain waits on the whole
    vector clock; spread its waits across trailing sync-engine nops."""
    import concourse.tile as tile
    import concourse.mybir as mybir
    from concourse.tile import ScopedClock

    if getattr(tile.TileContext, "_drain_patch_installed", False):
        return

    def _patched(self, tick_clock, wait_clock):
        nc = self.nc
        drain_inst = nc.sync.drain()
        wait_clock.add_sem_waits(
            drain_inst.ins, ScopedClock({None: tick_clock.global_clock})
        )
        si = drain_inst.ins.sync_info
        waits = list(si.on_wait or []) if si is not None else []
        if len(waits) > 1:
            si.on_wait = waits[:1]
            for w in waits[1:]:
                nop = nc.sync.nop(nofuse=True)
                nsi = nop.ins.sync_info
                if nsi is None:
                    nop.ins.sync_info = mybir.SyncInfo(on_wait=[w], on_update=[])
                else:
                    nsi.on_wait = [w]
        nc.all_engine_barrier()
        assert self.sems is not None
        popped = nc._tile_sem_poison_stack.pop()
        assert popped is self._sem_poison
        nc.clear_and_free_semaphores(list(self.sems.allocated().values()))
        nc.all_engine_barrier()

    tile.TileContext._drain_and_barrier = _patched
    tile.TileContext._drain_patch_installed = True


def _split_excess_waits(nc, maxw=1):
    """walrus's per-instruction sync-wait encodings hold few waits; hoist
    excess waits onto same-engine nops spliced immediately before the
    overloaded instruction (adjacent on the same queue, so ordering
    semantics are unchanged)."""
    import copy
    import concourse.mybir as mybir

    ctr = [0]
    # a genuine InstNoOp prototype (left at stream end, harmless)
    proto = nc.sync.nop(nofuse=True)
    _NOP_PROTO = copy.deepcopy(proto.ins)
    _NOP_PROTO.sync_info = None

    def make_nop(proto_engine, waits):
        ctr[0] += 1
        nop = copy.deepcopy(_NOP_PROTO)
        nop.name = f"I-waitsplit-{ctr[0]}"
        nop.engine = proto_engine
        nop.sync_info = mybir.SyncInfo(on_wait=list(waits), on_update=[])
        return nop

    for bb in nc.m.functions[0].blocks:
        changed = False
        out = []
        for inst in bb.instructions:
            si = inst.sync_info
            waits = list(si.on_wait) if (si is not None and si.on_wait) else []
            if len(waits) > maxw and type(inst).__name__ != "InstISA":
                for i in range(0, len(waits) - maxw, maxw):
                    out.append(make_nop(inst.engine, waits[i:i + maxw]))
                si.on_wait = waits[len(waits) - maxw:]
                changed = True
            out.append(inst)
        if changed:
            bb.instructions = out
    return nc


def _build_program(s_deq):
    """Build the per-core Bass program (identical across cores; data-parallel).

    s_deq[l] = in_scale[l]*w_scale[l] as python floats (fp32-exact values)
    """
    import concourse.bass as bass
    import concourse.mybir as mybir
    import concourse.tile as tile

    _install_drain_patch()
    dt = mybir.dt
    Alu = mybir.AluOpType
    Act = mybir.ActivationFunctionType

    nc = bass.Bass()
    W_d = nc.dram_tensor("wt", [NUM_LAYERS, KT, P, KT, P], dt.bfloat16,
                         kind="ExternalInput")
    X_d = nc.dram_tensor("xq0", [KT, P, NLOC], dt.bfloat16, kind="ExternalInput")
    G_d = nc.dram_tensor("gam", [NUM_LAYERS - 1, P, KT], dt.float32,
                         kind="ExternalInput")
    B_d = nc.dram_tensor("bet", [NUM_LAYERS - 1, P, KT], dt.float32,
                         kind="ExternalInput")
    O_d = nc.dram_tensor("out", [D, NLOC], dt.float32, kind="ExternalOutput")

    f32, f32r, bf16 = dt.float32, dt.float32r, dt.bfloat16

    with tile.TileContext(nc) as tc:
        with (
            tc.tile_pool(name="xq", bufs=64) as xq_pool,
            tc.tile_pool(name="h", bufs=33) as h_pool,
            tc.tile_pool(name="w", bufs=4) as w_pool,
            tc.tile_pool(name="sq", bufs=3) as sq_pool,
            tc.tile_pool(name="acc", bufs=4) as acc_pool,
            tc.tile_pool(name="ma", bufs=4) as ma_pool,
            tc.tile_pool(name="st", bufs=7) as st_pool,
            tc.tile_pool(name="str", bufs=3) as str_pool,
            tc.tile_pool(name="gb", bufs=6) as gb_pool,
            tc.tile_pool(name="const", bufs=1) as const_pool,
            tc.tile_pool(name="mmps", bufs=2, space="PSUM") as mm_ps,
            tc.tile_pool(name="stps", bufs=2, space="PSUM") as st_ps,
            tc.tile_pool(name="bcps", bufs=4, space="PSUM") as bc_ps,
        ):
            ones_f = const_pool.tile([P, 1], f32)
            nc.vector.memset(ones_f[:], 1.0)
            ones = const_pool.tile([P, 1], f32r)
            nc.vector.tensor_copy(ones[:], ones_f[:])
            eps = const_pool.tile([1, 1], f32)
            nc.vector.memset(eps[:], 1e-5)
            ones_row_f = const_pool.tile([1, P], f32)
            nc.vector.memset(ones_row_f[:], 1.0)
            ones_row = const_pool.tile([1, P], f32r)
            nc.vector.tensor_copy(ones_row[:], ones_row_f[:])

            # critical-path weight prefetch for the very first tiles, ahead
            # of the 8MB xq0 bulk DMA
            pre_w = {}
            for ot in range(2):
                t = w_pool.tile([P, KT, P], bf16, tag="w")
                nc.sync.dma_start(t[:], W_d[0, ot])
                pre_w[ot] = t

            xq_tiles = {}
            for half in range(2):
                for kt in range(KT):
                    t = xq_pool.tile([P, HALF], bf16, tag="xq")
                    nc.sync.dma_start(
                        t[:], X_d[kt, :, half * HALF:(half + 1) * HALF])
                    xq_tiles[(0, half, kt)] = t

            # gamma' = gamma/in_scale[l+1], beta' = beta/in_scale[l+1],
            # one strided DMA per layer each: [P, KT] tiles
            G1, B1 = [], []
            for l in range(NUM_LAYERS - 1):
                g = gb_pool.tile([P, KT], f32, tag="gb")
                nc.sync.dma_start(g[:], G_d[l])
                G1.append(g)
                b = gb_pool.tile([P, KT], f32, tag="gb")
                nc.sync.dma_start(b[:], B_d[l])
                B1.append(b)

            h_tiles = {}

            def emit_stats(pv):
                """Partition-reduce the accumulated sums; tiny row math."""
                l, half, accS, accQ = pv["l"], pv["half"], pv["accS"], pv["accQ"]
                S_ps = st_ps.tile([1, HALF], f32, tag="stps")
                nc.tensor.matmul(S_ps[:], ones[:], accS[:],
                                 start=True, stop=True, skip_group_check=True)
                Q_ps = st_ps.tile([1, HALF], f32, tag="stps")
                nc.tensor.matmul(Q_ps[:], ones[:], accQ[:],
                                 start=True, stop=True, skip_group_check=True)
                mu = st_pool.tile([1, HALF], f32, tag="st")
                nc.vector.tensor_scalar_mul(mu[:], S_ps[:], 1.0 / D)
                q = st_pool.tile([1, HALF], f32, tag="st")
                nc.vector.tensor_scalar_mul(q[:], Q_ps[:], 1.0 / D)
                var = st_pool.tile([1, HALF], f32, tag="st")
                nc.vector.tensor_tensor(var[:], mu[:], mu[:], op=Alu.mult)
                nc.vector.tensor_tensor(var[:], q[:], var[:], op=Alu.subtract)
                std = st_pool.tile([1, HALF], f32, tag="st")
                nc.scalar.activation(std[:], var[:], Act.Sqrt, bias=eps[:])
                rstd = st_pool.tile([1, HALF], f32, tag="st")
                nc.vector.reciprocal(rstd[:], std[:])
                nmr = st_pool.tile([1, HALF], f32, tag="st")
                nc.vector.scalar_tensor_tensor(
                    nmr[:], mu[:], -1.0, rstd[:], op0=Alu.mult, op1=Alu.mult)
                rstd_r = str_pool.tile([1, HALF], f32r, tag="str")
                nc.vector.tensor_copy(rstd_r[:], rstd[:])
                nmr_r = str_pool.tile([1, HALF], f32r, tag="str")
                nc.vector.tensor_copy(nmr_r[:], nmr[:])
                pv["rstd_r"], pv["nmr_r"] = rstd_r, nmr_r

            def emit_post(pv):
                """Broadcast rows across partitions, then normalize +
                requantize the 32 held h tiles into next layer's xq."""
                l, half = pv["l"], pv["half"]
                rstdB = bc_ps.tile([P, HALF], f32, tag="bcps")
                nc.tensor.matmul(rstdB[:], ones_row[:], pv["rstd_r"][:],
                                 start=True, stop=True, skip_group_check=True)
                muB = bc_ps.tile([P, HALF], f32, tag="bcps")
                nc.tensor.matmul(muB[:], ones_row[:], pv["nmr_r"][:],
                                 start=True, stop=True, skip_group_check=True)
                for ft in range(KT):
                    h_t = h_tiles.pop((half, ft))
                    m1 = ma_pool.tile([P, HALF], f32, tag="ma")
                    nc.vector.tensor_scalar_mul(m1[:], rstdB[:],
                                                G1[l][:, ft:ft + 1])
                    a1 = ma_pool.tile([P, HALF], f32, tag="ma")
                    nc.vector.tensor_scalar(a1[:], muB[:],
                                            G1[l][:, ft:ft + 1],
                                            B1[l][:, ft:ft + 1],
                                            op0=Alu.mult, op1=Alu.add)
                    nc.vector.tensor_tensor(h_t[:], h_t[:], m1[:], op=Alu.mult)
                    nc.vector.tensor_tensor(h_t[:], h_t[:], a1[:], op=Alu.add)
                    # + MAGIC: the fp32 write rounds to integer (RNE)
                    nc.vector.tensor_scalar_add(h_t[:], h_t[:], MAGIC)
                    xq_t = xq_pool.tile([P, HALF], bf16, tag="xq")
                    nc.vector.tensor_scalar(xq_t[:], h_t[:], MAGIC + 127.0,
                                            MAGIC, op0=Alu.min,
                                            op1=Alu.subtract)
                    nc.vector.tensor_scalar_max(xq_t[:], xq_t[:], -128.0)
                    xq_tiles[(l + 1, half, ft)] = xq_t

            def emit_phase(l, half, prev):
                last = l == NUM_LAYERS - 1
                if not last:
                    accS = acc_pool.tile([P, HALF], f32r, tag="acc")
                    accQ = acc_pool.tile([P, HALF], f32r, tag="acc")
                for ot in range(KT):
                    if l == 0 and half == 0 and ot in pre_w:
                        w = pre_w[ot]
                    else:
                        w = w_pool.tile([P, KT, P], bf16, tag="w")
                        nc.sync.dma_start(w[:], W_d[l, ot])
                    ps = mm_ps.tile([P, HALF], f32, tag="mmps")
                    for kt in range(KT):
                        nc.tensor.matmul(
                            ps[:], w[:, kt, :], xq_tiles[(l, half, kt)][:],
                            start=(kt == 0), stop=(kt == KT - 1),
                            skip_group_check=True)
                    h_t = h_pool.tile([P, HALF], f32, tag="h")
                    if not last:
                        nc.scalar.activation(h_t[:], ps[:], Act.Relu,
                                             scale=float(s_deq[l]))
                        sq = sq_pool.tile([P, HALF], f32, tag="sq")
                        nc.vector.tensor_tensor(sq[:], h_t[:], h_t[:],
                                                op=Alu.mult)
                        if ot == 0:
                            nc.vector.tensor_copy(accS[:], h_t[:])
                            nc.vector.tensor_copy(accQ[:], sq[:])
                        else:
                            nc.vector.tensor_tensor(
                                accS[:], accS[:].bitcast(f32), h_t[:],
                                op=Alu.add)
                            nc.vector.tensor_tensor(
                                accQ[:], accQ[:].bitcast(f32), sq[:],
                                op=Alu.add)
                        h_tiles[(half, ot)] = h_t
                    else:
                        nc.scalar.activation(h_t[:], ps[:], Act.Copy,
                                             scale=float(s_deq[l]))
                        nc.sync.dma_start(
                            O_d[ot * P:(ot + 1) * P,
                                half * HALF:(half + 1) * HALF], h_t[:])
                    if ot == 1 and prev is not None:
                        emit_stats(prev)
                    if ot == 3 and prev is not None:
                        emit_post(prev)
                if not last:
                    return {"l": l, "half": half, "accS": accS, "accQ": accQ}
                return None

            prev = None
            for l in range(NUM_LAYERS):
                for half in range(2):
                    prev = emit_phase(l, half, prev)

    return _split_excess_waits(nc)


def kernel(x, Ws, w_scales, in_scales, gammas, betas, _trace=False):
    import ml_dtypes
    from concourse.bass_utils import run_bass_kernel_spmd

    f32 = np.float32
    C = f32(MAGIC)
    x = np.asarray(x, f32)
    Ws = np.asarray(Ws, f32)
    w_scales = np.asarray(w_scales, f32)
    in_scales = np.asarray(in_scales, f32)
    gammas = np.asarray(gammas, f32)
    betas = np.asarray(betas, f32)

    # ---- host prep (offline-weight-style preprocessing) ----
    # ternary quantize weights; XLA divides by reciprocal-multiply and
    # rounds nearest-even, both reproduced here bitwise.
    WT = np.empty((NUM_LAYERS, KT, P, KT, P), ml_dtypes.bfloat16)
    for l in range(NUM_LAYERS):
        wq = ((Ws[l] * (f32(1.0) / w_scales[l])) + C) - C
        wq = np.clip(wq, -1.0, 1.0).astype(f32)
        # WT[l, ot, kp, kt, o] = wq[ot*128+o, kt*128+kp]
        t = wq.reshape(KT, P, KT, P)          # [ot, o, kt, kp]
        WT[l] = t.transpose(0, 3, 2, 1).astype(ml_dtypes.bfloat16)

    xq0 = ((x * (f32(1.0) / in_scales[0])) + C) - C
    xq0 = np.clip(xq0, -128.0, 127.0).astype(f32)
    xT = np.ascontiguousarray(xq0.T)           # [k, n]

    # gamma' = gamma/in_scale[l+1], beta' = beta/in_scale[l+1],
    # laid out [layer, partition, feature-tile] for single-DMA loads
    inv_in = [f32(1.0) / in_scales[l] for l in range(NUM_LAYERS)]
    G = np.empty((NUM_LAYERS - 1, P, KT), f32)
    B = np.empty((NUM_LAYERS - 1, P, KT), f32)
    for l in range(NUM_LAYERS - 1):
        G[l] = (gammas[l] * inv_in[l + 1]).reshape(KT, P).T
        B[l] = (betas[l] * inv_in[l + 1]).reshape(KT, P).T

    s_deq = [float(in_scales[l] * w_scales[l]) for l in range(NUM_LAYERS)]

    key = tuple(s_deq)
    if key not in _prog_cache:
        _prog_cache[key] = _build_program(s_deq)
    nc = _prog_cache[key]

    in_maps = []
    for c in range(NUM_CORES):
        xs = xT[:, c * NLOC:(c + 1) * NLOC].reshape(KT, P, NLOC)
        in_maps.append({
            "wt": WT,
            "xq0": np.ascontiguousarray(xs).astype(ml_dtypes.bfloat16),
            "gam": G,
            "bet": B,
        })

    res = run_bass_kernel_spmd(nc, in_maps, list(range(NUM_CORES)),
                               trace=_trace)
    if _trace:
        kernel.last_exec_ns = res.exec_time_ns

    outT = np.concatenate(
        [res.results[c]["out"] for c in range(NUM_CORES)], axis=1)
    return np.ascontiguousarray(outT.T).astype(np.float32)


kernel.last_exec_ns = None
